# revision 31
# baseline (speedup 1.0000x reference)
"""Causal attention (B=4096, T=64, C=64) on 8 TRN2 NeuronCores, pure data parallel.

Per core: x shard [512, 64, 64]. 512-token tiles (8 batches), bf16 matmuls with
fused weights A=Wq^T Wk, B=Wv^T Wp^T, bias folded into xB.

Layout: x loaded permuted (partition p <- tokens 4p..4p+3, 1KB descriptors).
8 PE transposes per tile (K=64 halves at tile positions (0,0)/(64,64)) write
strided PSUM columns -> xT [128 parts = 2 token-halves x 64c, 256 tok] in
natural token order. hT = A^T@xT, scores = xT_b^T@hT_b (+causal mask matmul),
exp on ACT, xB = xT_b^T@B + bp, y split into even/odd-token matmuls so each
PSUM partition holds 2 consecutive tokens -> 512B output descriptors.
Output DMA on the ACT HWDGE queue; input DMA on SP; 4-tile DMA chunks.
Elementwise: ACT(hT copy, exp), DVE(xB+bias, recip, yscale lo), Pool(xT copy,
yscale hi).
"""

import numpy as np

import concourse.bass as bass
import concourse.mybir as mybir
import concourse.tile as tile
import concourse.masks as masks
from concourse import bacc

F32 = mybir.dt.float32
F32R = mybir.dt.float32r
BF16 = mybir.dt.bfloat16
ADD = mybir.AluOpType.add
MULT = mybir.AluOpType.mult

N_CORES = 8
B, T, C = 4096, 64, 64
B_LOC = B // N_CORES  # 512 batches per core

MASK_VAL = -1e9


def build_nc(b_loc=B_LOC, batches_per_tile=8, reps=1):
    """Build the single-core Bass graph (SPMD: same graph on all 8 cores)."""
    assert batches_per_tile == 8
    TOK = batches_per_tile * T          # tokens per tile (512)
    n_tiles = b_loc // batches_per_tile  # 64
    TPC = 2                              # tiles per DMA chunk
    n_chunks = n_tiles // TPC            # 16
    HTOK = TOK // 2                      # 256 tokens per half

    nc = bacc.Bacc("TRN2", target_bir_lowering=False, debug=False)

    x_ext = nc.declare_dram_parameter("x", [b_loc, T, C], F32, isOutput=False)
    Wk_ext = nc.declare_dram_parameter("Wk", [C, C], F32, isOutput=False)
    Wq_ext = nc.declare_dram_parameter("Wq", [C, C], F32, isOutput=False)
    Wv_ext = nc.declare_dram_parameter("Wv", [C, C], F32, isOutput=False)
    Wp_ext = nc.declare_dram_parameter("Wp", [C, C], F32, isOutput=False)
    bp_ext = nc.declare_dram_parameter("bp", [C], F32, isOutput=False)
    out_ext = nc.declare_dram_parameter("out", [b_loc, T, C], F32, isOutput=True)

    x_flat = x_ext.ap().rearrange("b t c -> (b t) c")
    out_flat = out_ext.ap().rearrange("b t c -> (b t) c")

    # maskT[s, t] = 0 where s <= t else MASK_VAL*8 (exp scale 0.125 applied after)
    m1 = np.where(
        np.arange(T)[:, None] <= np.arange(T)[None, :], 0.0, MASK_VAL * 8.0
    ).astype(np.float32)
    maskT2_dram = nc.inline_tensor(np.vstack([m1, m1]), name="maskT2_const")
    i64 = np.eye(64, dtype=np.float32)
    ident2_dram = nc.inline_tensor(np.vstack([i64, i64]), name="ident2_const")

    with tile.TileContext(nc) as tc:
        with (
            tc.tile_pool(name="const", bufs=1) as constp,
            tc.tile_pool(name="xin", bufs=2) as xin_pool,
            tc.tile_pool(name="work", bufs=3) as work_pool,
            tc.tile_pool(name="yout", bufs=2) as yout_pool,
            tc.tile_pool(name="ps", bufs=2, space="PSUM") as ps,
        ):
            # ---- one-time constants ----
            ident2 = constp.tile([128, 64], F32)
            nc.sync.dma_start(ident2[:], ident2_dram.ap())
            ident2_bf = constp.tile([128, 64], BF16)
            nc.vector.tensor_copy(ident2_bf[:], ident2[:])
            ident_bf = constp.tile([128, 128], BF16)
            masks.make_identity(nc, ident_bf[:])
            maskT2f = constp.tile([128, T], F32)
            nc.sync.dma_start(maskT2f[:], maskT2_dram.ap())
            maskT2 = constp.tile([128, T], BF16)
            nc.vector.tensor_copy(maskT2[:], maskT2f[:])

            # weights: natural DMA (contiguous), fused A = Wq^T Wk, B = Wv^T Wp^T
            wnat = constp.tile([C, 4 * C], F32)
            for i, w_ext in enumerate((Wq_ext, Wk_ext, Wv_ext, Wp_ext)):
                nc.sync.dma_start(wnat[:, i * C : (i + 1) * C], w_ext.ap())
            # Wp^T via PE transpose (f32)
            wT_ps = ps.tile([C, C], F32, tag="y")
            nc.tensor.transpose(
                wT_ps[:], wnat[:, 3 * C : 4 * C], ident2[0:C, 0:C]
            )
            WpTf = constp.tile([C, C], F32)
            nc.vector.tensor_copy(WpTf[:], wT_ps[:])
            # A and B, each replicated on both partition halves
            AB_ps = ps.tile([128, 2, C], F32, tag="xh")
            for h in (0, 1):
                nc.tensor.matmul(
                    AB_ps[h * C : (h + 1) * C, 0, :],
                    wnat[:, 0:C], wnat[:, C : 2 * C],
                    tile_position=(0, h * C),
                )
                nc.tensor.matmul(
                    AB_ps[h * C : (h + 1) * C, 1, :],
                    wnat[:, 2 * C : 3 * C], WpTf[:],
                    tile_position=(0, h * C),
                )
            A2 = constp.tile([128, C], BF16)
            nc.vector.tensor_copy(A2[:], AB_ps[:, 0, :])
            B2 = constp.tile([128, C], BF16)
            nc.vector.tensor_copy(B2[:], AB_ps[:, 1, :])

            # bias broadcast to [128, C] via ones-matmul (K=1)
            bp_row = constp.tile([1, C], F32)
            nc.sync.dma_start(bp_row[:], bp_ext.ap().unsqueeze(0))
            ones_row128 = constp.tile([1, 128], F32)
            nc.vector.memset(ones_row128[:], 1.0)
            bias_ps = ps.tile([128, C], F32, tag="wb")
            nc.tensor.matmul(bias_ps[:], ones_row128[:], bp_row[:])
            bias_bc = constp.tile([128, C], F32)
            nc.vector.tensor_copy(bias_bc[:], bias_ps[:])



            rep_ctx = tc.For_i(0, reps, 1) if reps > 1 else None
            if rep_ctx is not None:
                rep_ctx.__enter__()

            # --- software-pipelined emission (skewed): per iteration i the
            # PE queue is [hT(i), xB(i), T(i+1), S(i), y(i-1)] so that every
            # cross-engine handoff is covered by independent PE work ---
            S = [dict() for _ in range(n_tiles)]
            chunks = {}

            def emit_load(t):
                st, u = divmod(t, TPC)
                if u == 0:
                    x4 = xin_pool.tile([128, TPC, 4 * C], F32, tag="x4")
                    base = st * TPC * TOK
                    nc.sync.dma_start(
                        x4[:],
                        x_flat[base : base + TPC * TOK, :].rearrange(
                            "(u p m) c -> p u (m c)", u=TPC, p=128, m=4
                        ),
                    )
                    y4 = yout_pool.tile([128, TPC, 2, 2, C], F32, tag="y4")
                    chunks[st] = (x4, y4)
                x4, y4 = chunks[st]
                s = S[t]
                s["y4"], s["u"] = y4, u
                x_bf = work_pool.tile([128, 4 * C], BF16, tag="x_bf")
                nc.gpsimd.tensor_copy(x_bf[:], x4[:, u, :])
                xh_ps = ps.tile([128, 2, HTOK // 4, 4], F32, tag="xh")
                for h in (0, 1):
                    for m in range(4):
                        nc.tensor.matmul(
                            xh_ps[h * C : (h + 1) * C, 0, :, m],
                            x_bf[h * C : (h + 1) * C, m * C : (m + 1) * C],
                            ident2_bf[h * C : (h + 1) * C, :],
                            tile_position=(h * C, h * C),
                        )
                xT = work_pool.tile([128, HTOK], BF16, tag="xT")
                nc.scalar.copy(
                    xT[:, 0 : HTOK // 2],
                    xh_ps[:, 0, 0 : HTOK // 8, :].rearrange("p a m -> p (a m)"),
                )
                nc.vector.tensor_copy(
                    xT[:, HTOK // 2 :],
                    xh_ps[:, 0, HTOK // 8 :, :].rearrange("p a m -> p (a m)"),
                )
                s["xh_ps"], s["xT"] = xh_ps, xT

            def emit_hT(t):
                s = S[t]
                xh_ps, xT = s["xh_ps"], s["xT"]
                for h in (0, 1):
                    nc.tensor.matmul(
                        xh_ps[h * C : (h + 1) * C, 1].rearrange(
                            "p a m -> p (a m)"
                        ),
                        A2[h * C : (h + 1) * C, :],
                        xT[h * C : (h + 1) * C, :],
                        tile_position=(h * C, h * C),
                    )
                hT = work_pool.tile([128, HTOK], BF16, tag="hT")
                nc.scalar.copy(hT[:], xh_ps[:, 1].rearrange("p a m -> p (a m)"))
                s["hT"] = hT

            def emit_xB(t):
                s = S[t]
                xT = s["xT"]
                wb_ps = ps.tile([128, 2, 4, T], F32, tag="wb")
                for h in (0, 1):
                    for bl in range(4):
                        nc.tensor.matmul(
                            wb_ps[h * C : (h + 1) * C, 1, bl, :],
                            xT[h * C : (h + 1) * C, bl * T : (bl + 1) * T],
                            B2[h * C : (h + 1) * C, :],
                            tile_position=(h * C, h * C),
                        )
                xB = work_pool.tile([128, 4, C + 1], BF16, tag="xB")
                nc.vector.tensor_tensor(
                    xB[:, :, 0:C],
                    wb_ps[:, 1],
                    bias_bc[:].unsqueeze(1).broadcast_to([128, 4, C]),
                    ADD,
                )
                nc.vector.memset(xB[:, :, C : C + 1], 1.0)
                s["wb_ps"], s["xB"] = wb_ps, xB

            def emit_S(t):
                s = S[t]
                xT, hT, wb_ps = s["xT"], s["hT"], s["wb_ps"]
                nc.tensor.matmul(
                    wb_ps[:, 0],
                    ident_bf[:],
                    maskT2[:].unsqueeze(1).broadcast_to([128, 4, T]),
                    start=True, stop=False, skip_group_check=True,
                )
                for h in (0, 1):
                    for bl in range(4):
                        nc.tensor.matmul(
                            wb_ps[h * C : (h + 1) * C, 0, bl, :],
                            xT[h * C : (h + 1) * C, bl * T : (bl + 1) * T],
                            hT[h * C : (h + 1) * C, bl * T : (bl + 1) * T],
                            start=False, stop=(h == 1 and bl == 3),
                            tile_position=(h * C, h * C),
                            skip_group_check=True,
                        )
                weiT_e = work_pool.tile([128, 4, T], BF16, tag="weiT_e")
                nc.scalar.activation(
                    weiT_e[:], wb_ps[:, 0],
                    mybir.ActivationFunctionType.Exp,
                    scale=0.125,
                )
                s["weiT_e"] = weiT_e

            def emit_y(t):
                s = S[t]
                weiT_e, xB, y4, u = s["weiT_e"], s["xB"], s["y4"], s["u"]
                y_ps = ps.tile([128, 2, 4, 2 * C], F32, tag="y")
                for h in (0, 1):
                    for bl in range(4):
                        for par in (0, 1):
                            nc.tensor.matmul(
                                y_ps[bl * 32 : (bl + 1) * 32, h, par, 0 : C + 1],
                                weiT_e[h * C : (h + 1) * C, bl, par::2],
                                xB[h * C : (h + 1) * C, bl, :],
                                tile_position=(h * C, bl * 32),
                            )
                recip = work_pool.tile([128, 2, 2], F32, tag="recip")
                nc.vector.reciprocal(recip[:], y_ps[:, :, 0:2, C : C + 1])
                nc.vector.tensor_tensor(
                    y4[:, u, :, :, :],
                    y_ps[:, :, 0:2, 0:C],
                    recip[:].unsqueeze(3).broadcast_to([128, 2, 2, C]),
                    MULT,
                )
                S[t] = {}
                if u == TPC - 1:
                    st = t // TPC
                    base = st * TPC * TOK
                    nc.sync.dma_start(
                        out_flat[base : base + TPC * TOK, :].rearrange(
                            "(u j p m) c -> p u j (m c)", u=TPC, j=2, p=128, m=2
                        ),
                        y4[:].rearrange("p u j m c -> p u j (m c)"),
                    )

            emit_load(0)
            for t in range(n_tiles):
                emit_hT(t)
                emit_xB(t)
                if t + 1 < n_tiles:
                    emit_load(t + 1)
                emit_S(t)
                if t > 0:
                    emit_y(t - 1)
            emit_y(n_tiles - 1)

            if rep_ctx is not None:
                rep_ctx.__exit__(None, None, None)

    nc.compile()
    return nc


_NC_CACHE = {}


def _get_nc(b_loc, batches_per_tile=8):
    key = (b_loc, batches_per_tile)
    if key not in _NC_CACHE:
        _NC_CACHE[key] = build_nc(b_loc, batches_per_tile)
    return _NC_CACHE[key]


def kernel(x, Wk, Wq, Wv, Wp, bp):
    from concourse.bass_utils import run_bass_kernel_spmd

    x = np.ascontiguousarray(x, dtype=np.float32)
    weights = {
        "Wk": np.ascontiguousarray(Wk, dtype=np.float32),
        "Wq": np.ascontiguousarray(Wq, dtype=np.float32),
        "Wv": np.ascontiguousarray(Wv, dtype=np.float32),
        "Wp": np.ascontiguousarray(Wp, dtype=np.float32),
        "bp": np.ascontiguousarray(bp, dtype=np.float32),
    }
    nc = _get_nc(B_LOC)
    in_maps = [
        {"x": x[i * B_LOC : (i + 1) * B_LOC], **weights} for i in range(N_CORES)
    ]
    res = run_bass_kernel_spmd(nc, in_maps, core_ids=list(range(N_CORES)))
    outs = [res.results[i]["out"] for i in range(N_CORES)]
    return np.concatenate(outs, axis=0)


# revision 39
# speedup vs baseline: 1.6152x; 1.6152x over previous
"""Causal attention (B=4096, T=64, C=64) on 8 TRN2 NeuronCores, pure data parallel.

Per core: x shard [512, 64, 64]. 512-token tiles (8 batches), bf16 matmuls with
fused weights A=Wq^T Wk, B=Wv^T Wp^T, bias folded into xB.

Layout: x loaded permuted (partition p <- tokens 4p..4p+3, 1KB descriptors).
8 PE transposes per tile (K=64 halves at tile positions (0,0)/(64,64)) write
strided PSUM columns -> xT [128 parts = 2 token-halves x 64c, 256 tok] in
natural token order. hT = A^T@xT, scores = xT_b^T@hT_b (+causal mask matmul),
exp on ACT, xB = xT_b^T@B + bp, y split into even/odd-token matmuls so each
PSUM partition holds 2 consecutive tokens -> 512B output descriptors.
Output DMA on the ACT HWDGE queue; input DMA on SP; 4-tile DMA chunks.
Elementwise: ACT(hT copy, exp), DVE(xB+bias, recip, yscale lo), Pool(xT copy,
yscale hi).
"""

import numpy as np

import concourse.bass as bass
import concourse.mybir as mybir
import concourse.tile as tile
import concourse.masks as masks
from concourse import bacc

F32 = mybir.dt.float32
F32R = mybir.dt.float32r
BF16 = mybir.dt.bfloat16
ADD = mybir.AluOpType.add
MULT = mybir.AluOpType.mult

N_CORES = 8
B, T, C = 4096, 64, 64
B_LOC = B // N_CORES  # 512 batches per core

MASK_VAL = -1e9


def build_nc(b_loc=B_LOC, batches_per_tile=8, reps=1):
    """Build the single-core Bass graph (SPMD: same graph on all 8 cores)."""
    assert batches_per_tile == 8
    TOK = batches_per_tile * T          # tokens per tile (512)
    n_tiles = b_loc // batches_per_tile  # 64
    TPC = 2                              # tiles per DMA chunk
    n_chunks = n_tiles // TPC            # 16
    HTOK = TOK // 2                      # 256 tokens per half

    nc = bacc.Bacc("TRN2", target_bir_lowering=False, debug=False)

    x_ext = nc.declare_dram_parameter("x", [b_loc, T, C], F32, isOutput=False)
    Wk_ext = nc.declare_dram_parameter("Wk", [C, C], F32, isOutput=False)
    Wq_ext = nc.declare_dram_parameter("Wq", [C, C], F32, isOutput=False)
    Wv_ext = nc.declare_dram_parameter("Wv", [C, C], F32, isOutput=False)
    Wp_ext = nc.declare_dram_parameter("Wp", [C, C], F32, isOutput=False)
    bp_ext = nc.declare_dram_parameter("bp", [C], F32, isOutput=False)
    out_ext = nc.declare_dram_parameter("out", [b_loc, T, C], F32, isOutput=True)

    x_flat = x_ext.ap().rearrange("b t c -> (b t) c")
    out_flat = out_ext.ap().rearrange("b t c -> (b t) c")

    # maskT[s, t] = 0 where s <= t else MASK_VAL*8 (exp scale 0.125 applied after)
    m1 = np.where(
        np.arange(T)[:, None] <= np.arange(T)[None, :], 0.0, MASK_VAL * 8.0
    ).astype(np.float32)
    maskT2_dram = nc.inline_tensor(np.vstack([m1, m1]), name="maskT2_const")
    i64 = np.eye(64, dtype=np.float32)
    ident2_dram = nc.inline_tensor(np.vstack([i64, i64]), name="ident2_const")

    with tile.TileContext(nc) as tc:
        with (
            tc.tile_pool(name="const", bufs=1) as constp,
            tc.tile_pool(name="xin", bufs=2) as xin_pool,
            tc.tile_pool(name="work", bufs=3) as work_pool,
            tc.tile_pool(name="yout", bufs=2) as yout_pool,
            tc.tile_pool(name="ps", bufs=2, space="PSUM") as ps,
        ):
            # ---- one-time constants ----
            ident2 = constp.tile([128, 64], F32)
            nc.sync.dma_start(ident2[:], ident2_dram.ap())
            ident2_bf = constp.tile([128, 64], BF16)
            nc.vector.tensor_copy(ident2_bf[:], ident2[:])
            ident_bf = constp.tile([128, 128], BF16)
            masks.make_identity(nc, ident_bf[:])
            maskT2f = constp.tile([128, T], F32)
            nc.sync.dma_start(maskT2f[:], maskT2_dram.ap())
            maskT2 = constp.tile([128, T], BF16)
            nc.vector.tensor_copy(maskT2[:], maskT2f[:])

            # weights: natural DMA (contiguous), fused A = Wq^T Wk, B = Wv^T Wp^T
            wnat = constp.tile([C, 4 * C], F32)
            for i, w_ext in enumerate((Wq_ext, Wk_ext, Wv_ext, Wp_ext)):
                nc.sync.dma_start(wnat[:, i * C : (i + 1) * C], w_ext.ap())
            # Wp^T via PE transpose (f32)
            wT_ps = ps.tile([C, C], F32, tag="y")
            nc.tensor.transpose(
                wT_ps[:], wnat[:, 3 * C : 4 * C], ident2[0:C, 0:C]
            )
            WpTf = constp.tile([C, C], F32)
            nc.vector.tensor_copy(WpTf[:], wT_ps[:])
            # A and B, each replicated on both partition halves
            AB_ps = ps.tile([128, 2, C], F32, tag="xh")
            for h in (0, 1):
                nc.tensor.matmul(
                    AB_ps[h * C : (h + 1) * C, 0, :],
                    wnat[:, 0:C], wnat[:, C : 2 * C],
                    tile_position=(0, h * C),
                )
                nc.tensor.matmul(
                    AB_ps[h * C : (h + 1) * C, 1, :],
                    wnat[:, 2 * C : 3 * C], WpTf[:],
                    tile_position=(0, h * C),
                )
            A2 = constp.tile([128, C], BF16)
            nc.vector.tensor_copy(A2[:], AB_ps[:, 0, :])
            B2 = constp.tile([128, C], BF16)
            nc.vector.tensor_copy(B2[:], AB_ps[:, 1, :])

            # bias broadcast to [128, C] via ones-matmul (K=1)
            bp_row = constp.tile([1, C], F32)
            nc.sync.dma_start(bp_row[:], bp_ext.ap().unsqueeze(0))
            ones_row128 = constp.tile([1, 128], F32)
            nc.vector.memset(ones_row128[:], 1.0)
            bias_ps = ps.tile([128, C], F32, tag="wb")
            nc.tensor.matmul(bias_ps[:], ones_row128[:], bp_row[:])
            bias_bc = constp.tile([128, C], F32)
            nc.vector.tensor_copy(bias_bc[:], bias_ps[:])



            rep_ctx = tc.For_i(0, reps, 1) if reps > 1 else None
            if rep_ctx is not None:
                rep_ctx.__enter__()

            # --- software-pipelined emission (skewed): per iteration i the
            # PE queue is [hT(i), xB(i), T(i+1), S(i), y(i-1)] so that every
            # cross-engine handoff is covered by independent PE work ---
            S = [dict() for _ in range(n_tiles)]
            chunks = {}

            def emit_load(t):
                st, u = divmod(t, TPC)
                if u == 0:
                    x4 = xin_pool.tile([128, TPC, 4 * C], F32, tag="x4")
                    base = st * TPC * TOK
                    nc.sync.dma_start(
                        x4[:],
                        x_flat[base : base + TPC * TOK, :].rearrange(
                            "(u p m) c -> p u (m c)", u=TPC, p=128, m=4
                        ),
                    )
                    y4 = yout_pool.tile([128, TPC, 2, 2, C], F32, tag="y4")
                    chunks[st] = (x4, y4)
                x4, y4 = chunks[st]
                s = S[t]
                s["y4"], s["u"] = y4, u
                x_bf = work_pool.tile([128, 4 * C], BF16, tag="x_bf")
                nc.gpsimd.tensor_copy(x_bf[:], x4[:, u, :])
                xh_ps = ps.tile([128, 2, HTOK // 4, 4], F32, tag="xh")
                for h in (0, 1):
                    for m in range(4):
                        nc.tensor.matmul(
                            xh_ps[h * C : (h + 1) * C, 0, :, m],
                            x_bf[h * C : (h + 1) * C, m * C : (m + 1) * C],
                            ident2_bf[h * C : (h + 1) * C, :],
                            tile_position=(h * C, h * C),
                        )
                xT = work_pool.tile([128, HTOK], BF16, tag="xT")
                nc.scalar.copy(
                    xT[:, 0 : HTOK // 2],
                    xh_ps[:, 0, 0 : HTOK // 8, :].rearrange("p a m -> p (a m)"),
                )
                nc.vector.tensor_copy(
                    xT[:, HTOK // 2 :],
                    xh_ps[:, 0, HTOK // 8 :, :].rearrange("p a m -> p (a m)"),
                )
                s["xh_ps"], s["xT"] = xh_ps, xT

            def emit_hT(t):
                s = S[t]
                xh_ps, xT = s["xh_ps"], s["xT"]
                for h in (0, 1):
                    nc.tensor.matmul(
                        xh_ps[h * C : (h + 1) * C, 1].rearrange(
                            "p a m -> p (a m)"
                        ),
                        A2[h * C : (h + 1) * C, :],
                        xT[h * C : (h + 1) * C, :],
                        tile_position=(h * C, h * C),
                    )
                hT = work_pool.tile([128, HTOK], BF16, tag="hT")
                nc.scalar.copy(hT[:], xh_ps[:, 1].rearrange("p a m -> p (a m)"))
                s["hT"] = hT

            def emit_xB(t):
                s = S[t]
                xT = s["xT"]
                wb_ps = ps.tile([128, 2, 4, T], F32, tag="wb")
                for h in (0, 1):
                    for bl in range(4):
                        nc.tensor.matmul(
                            wb_ps[h * C : (h + 1) * C, 1, bl, :],
                            xT[h * C : (h + 1) * C, bl * T : (bl + 1) * T],
                            B2[h * C : (h + 1) * C, :],
                            tile_position=(h * C, h * C),
                        )
                xB = work_pool.tile([128, 4, C + 1], BF16, tag="xB")
                nc.vector.memset(xB[:, :, C : C + 1], 1.0)
                nc.vector.tensor_tensor(
                    xB[:, :, 0:C],
                    wb_ps[:, 1],
                    bias_bc[:].unsqueeze(1).broadcast_to([128, 4, C]),
                    ADD,
                )
                s["wb_ps"], s["xB"] = wb_ps, xB

            def emit_S(t):
                s = S[t]
                xT, hT, wb_ps = s["xT"], s["hT"], s["wb_ps"]
                nc.tensor.matmul(
                    wb_ps[:, 0],
                    ident_bf[:],
                    maskT2[:].unsqueeze(1).broadcast_to([128, 4, T]),
                    start=True, stop=False, skip_group_check=True,
                )
                for h in (0, 1):
                    for bl in range(4):
                        nc.tensor.matmul(
                            wb_ps[h * C : (h + 1) * C, 0, bl, :],
                            xT[h * C : (h + 1) * C, bl * T : (bl + 1) * T],
                            hT[h * C : (h + 1) * C, bl * T : (bl + 1) * T],
                            start=False, stop=(h == 1 and bl == 3),
                            tile_position=(h * C, h * C),
                            skip_group_check=True,
                        )
                weiT_e = work_pool.tile([128, 4, T], BF16, tag="weiT_e")
                nc.scalar.activation(
                    weiT_e[:], wb_ps[:, 0],
                    mybir.ActivationFunctionType.Exp,
                    scale=0.125,
                )
                s["weiT_e"] = weiT_e

            def emit_y(t):
                s = S[t]
                weiT_e, xB, y4, u = s["weiT_e"], s["xB"], s["y4"], s["u"]
                # 8 matmuls, one per batch; psum partition = token % 128.
                # One bank per K-half h (groups in a bank must share the K
                # row offset); slots 2..3 of dim 'a' are padding.
                y_ps = ps.tile([128, 2, 4, 2 * C], F32, tag="y")
                for h in (0, 1):
                    for bl in range(4):
                        po = C * (bl % 2)
                        nc.tensor.matmul(
                            y_ps[po : po + C, h, bl // 2, 0 : C + 1],
                            weiT_e[h * C : (h + 1) * C, bl, :],
                            xB[h * C : (h + 1) * C, bl, :],
                            tile_position=(h * C, po),
                        )
                recip = work_pool.tile([128, 2, 2], F32, tag="recip")
                nc.vector.reciprocal(recip[:], y_ps[:, :, 0:2, C : C + 1])
                nc.vector.tensor_tensor(
                    y4[:, u, :, :, :],
                    y_ps[:, :, 0:2, 0:C],
                    recip[:].unsqueeze(3).broadcast_to([128, 2, 2, C]),
                    MULT,
                )
                S[t] = {}
                # per-tile out DMA: 512 descriptors (the DGE ring caps ~512)
                base = t * TOK
                nc.sync.dma_start(
                    out_flat[base : base + TOK, :].rearrange(
                        "(h jj p) c -> p h jj c", h=2, jj=2, p=128
                    ),
                    y4[:, u, :, :, :],
                )

            emit_load(0)
            for t in range(n_tiles):
                emit_hT(t)
                emit_xB(t)
                if t + 1 < n_tiles:
                    emit_load(t + 1)
                emit_S(t)
                if t > 0:
                    emit_y(t - 1)
            emit_y(n_tiles - 1)

            if rep_ctx is not None:
                rep_ctx.__exit__(None, None, None)

    nc.compile()
    return nc


_NC_CACHE = {}


def _get_nc(b_loc, batches_per_tile=8):
    key = (b_loc, batches_per_tile)
    if key not in _NC_CACHE:
        _NC_CACHE[key] = build_nc(b_loc, batches_per_tile)
    return _NC_CACHE[key]


def kernel(x, Wk, Wq, Wv, Wp, bp):
    from concourse.bass_utils import run_bass_kernel_spmd

    x = np.ascontiguousarray(x, dtype=np.float32)
    weights = {
        "Wk": np.ascontiguousarray(Wk, dtype=np.float32),
        "Wq": np.ascontiguousarray(Wq, dtype=np.float32),
        "Wv": np.ascontiguousarray(Wv, dtype=np.float32),
        "Wp": np.ascontiguousarray(Wp, dtype=np.float32),
        "bp": np.ascontiguousarray(bp, dtype=np.float32),
    }
    nc = _get_nc(B_LOC)
    in_maps = [
        {"x": x[i * B_LOC : (i + 1) * B_LOC], **weights} for i in range(N_CORES)
    ]
    res = run_bass_kernel_spmd(nc, in_maps, core_ids=list(range(N_CORES)))
    outs = [res.results[i]["out"] for i in range(N_CORES)]
    return np.concatenate(outs, axis=0)
